# revision 4
# baseline (speedup 1.0000x reference)
"""Trainium2 Bass kernel for nn_DomainGCN (4-layer GCN + MLP head), 8 NeuronCores.

Strategy (graph/data parallel, per sharding hint):
  - Nodes sharded contiguously: core r owns rows [r*6272, (r+1)*6272) (padded).
  - Reformulation: Agg(h@W) with per-edge weight dis[src]*dis[dst] and
    self-loops folded in as extra edges (weight dis[j]^2), so each layer is
      z = h @ W            (dense, fp32r matmuls, feature-major h resident in SBUF)
      z -> bf16 -> AllGather (every core gets the full z, node-major, in DRAM)
      agg = S.T @ z[src]   (dma_gather of edge messages + one-hot-weighted
                            matmuls accumulating per 128-dst-node block in PSUM)
      h' = relu(agg + b)   (DVE add + ACT relu), PE-transposed back to
                            feature-major for the next dense matmul.
  - Edge bookkeeping (sort by dst block / src range, degree rsqrt, padding) is
    host-side index preprocessing; all FLOP-bearing math runs on device.
  - dma_gather indices are int16, so each block's edge list is split at
    src=32768 and gathered from a base-offset slice of the gathered z.
"""

import os
import math
import numpy as np

import concourse.bass as bass
import concourse.bacc as bacc
import concourse.mybir as mybir
import concourse.tile as tile
from concourse import bass_utils

# problem constants (hardcoded per task contract)
N, E = 50000, 800000
DIN = DH = 512
DE, MH, NCLS = 10, 64, 20
NCORES = 8
NB = 49                   # dst blocks of 128 nodes per core
SHARD = NB * 128          # 6272
NPAD = SHARD * NCORES     # 50176
SPLIT = 32768             # int16 gather-index boundary
KC = 4                    # 128-feature chunks of 512

f32 = mybir.dt.float32
f32r = mybir.dt.float32r
bf16 = mybir.dt.bfloat16
i16 = mybir.dt.int16

LAST_RESULT = None        # BassKernelResults of the most recent run (for test.py)
_BUILD_CACHE = {}


# ---------------------------------------------------------------- host prep

def _host_prep(x, edge_index):
    src = edge_index[0].astype(np.int64)
    dst = edge_index[1].astype(np.int64)
    deg = np.bincount(dst, minlength=N).astype(np.float32) + 1.0
    dis = (1.0 / np.sqrt(deg)).astype(np.float32)

    # augment with self loops; uniform edge weight dis[src]*dis[dst]
    sa = np.concatenate([src, np.arange(N, dtype=np.int64)])
    da = np.concatenate([dst, np.arange(N, dtype=np.int64)])
    w = (dis[sa] * dis[da]).astype(np.float32)

    r = da // SHARD
    b = (da % SHARD) // 128
    s = (sa >= SPLIT).astype(np.int64)
    order = np.lexsort((sa, s, b, r))
    sa, da, w = sa[order], da[order], w[order]
    key = (r * NB + b) * 2 + s
    ks = key[order]
    counts = np.bincount(ks, minlength=NCORES * NB * 2).reshape(NCORES, NB, 2)
    starts = np.zeros(NCORES * NB * 2 + 1, np.int64)
    np.cumsum(counts.reshape(-1), out=starts[1:])

    # uniform program structure: per (block, seg) tile count = max over cores
    T_seg = np.ceil(counts / 128.0).astype(np.int64).max(axis=0)  # [NB, 2]
    TT = int(T_seg.sum())

    gidx, dstc, wc = [], [], []
    for rr in range(NCORES):
        idx16 = np.zeros(TT * 128, np.int16)
        dcol = np.full(TT * 128, -1.0, np.float32)
        wcol = np.zeros(TT * 128, np.float32)
        cur = 0
        for bb in range(NB):
            for ss in range(2):
                k = (rr * NB + bb) * 2 + ss
                n = counts[rr, bb, ss]
                lo, hi = starts[k], starts[k] + n
                base = SPLIT if ss else 0
                idx16[cur:cur + n] = (sa[lo:hi] - base).astype(np.int16)
                dcol[cur:cur + n] = (da[lo:hi] - rr * SHARD - bb * 128).astype(np.float32)
                wcol[cur:cur + n] = w[lo:hi]
                cur += int(T_seg[bb, ss]) * 128   # pads: idx 0, dst -1, w 0
        gidx.append(np.tile(idx16.reshape(-1, 16).T, (8, 1)))       # [128, TT*8]
        dstc.append(np.ascontiguousarray(dcol.reshape(-1, 128).T))  # [128, TT]
        wc.append(np.ascontiguousarray(wcol.reshape(-1, 128).T))

    return {
        "T_seg": T_seg, "TT": TT, "gidx": gidx, "dstc": dstc, "wc": wc,
    }


def _chunk_w(W):
    """[K, M] -> [128, (K//128)*M] with k-chunk c at cols [c*M, (c+1)*M)."""
    K, M = W.shape
    return np.ascontiguousarray(
        W.reshape(K // 128, 128, M).transpose(1, 0, 2).reshape(128, -1)
    ).astype(np.float32)


# ---------------------------------------------------------------- kernel build

def _build(T_seg_t, TT):
    T_seg = np.asarray(T_seg_t).reshape(NB, 2)
    TBMAX = int((T_seg[:, 0] + T_seg[:, 1]).max())

    nc = bacc.Bacc("TRN2", target_bir_lowering=False, debug=False,
                   num_devices=NCORES)

    dt_in = {}

    def din(name, shape, dt):
        dt_in[name] = nc.dram_tensor(name, shape, dt, kind="ExternalInput")
        return dt_in[name]

    xT = din("xT", [DIN, SHARD], f32r)
    Wd = {l: din(f"W{l}", [128, KC * (DH if l < 4 else 128)], f32r) for l in (1, 2, 3, 4)}
    Bd = {l: din(f"B{l}", [128, DH if l < 4 else 128], f32) for l in (1, 2, 3, 4)}
    M1p = din("M1p", [128, MH], f32r)
    M2d = din("M2d", [MH, MH], f32r)
    M3d = din("M3d", [MH, NCLS], f32r)
    MB1 = din("MB1", [MH, 1], f32)
    MB2 = din("MB2", [MH, 1], f32)
    MB3b = din("MB3b", [128, NCLS], f32)
    iota_c = din("iota_c", [128, 128], f32)
    ident_c = din("ident_c", [128, 128], f32)
    gidx = din("gidx", [128, TT * 8], i16)
    dstc = din("dstc", [128, TT], f32)
    wc = din("wc", [128, TT], f32)
    out = nc.dram_tensor("out", [SHARD, NCLS], f32, kind="ExternalOutput")

    # persistent SBUF
    hT = [nc.alloc_sbuf_tensor(f"hT{k}", [128, SHARD], f32r).ap() for k in range(KC)]
    W_sb = nc.alloc_sbuf_tensor("W_sb", [128, KC * DH], f32r).ap()
    W4_sb = nc.alloc_sbuf_tensor("W4_sb", [128, KC * 128], f32r).ap()
    iota_sb = nc.alloc_sbuf_tensor("iota_sb", [128, 128], f32).ap()
    ident_sb = nc.alloc_sbuf_tensor("ident_sb", [128, 128], f32).ap()
    Bb_sb = {l: nc.alloc_sbuf_tensor(f"Bb{l}", [128, DH if l < 4 else 128], f32).ap()
             for l in (1, 2, 3, 4)}
    M1_sb = nc.alloc_sbuf_tensor("M1_sb", [128, MH], f32r).ap()
    M2_sb = nc.alloc_sbuf_tensor("M2_sb", [MH, MH], f32r).ap()
    M3_sb = nc.alloc_sbuf_tensor("M3_sb", [MH, NCLS], f32r).ap()
    MB1_sb = nc.alloc_sbuf_tensor("MB1_sb", [MH, 1], f32).ap()
    MB2_sb = nc.alloc_sbuf_tensor("MB2_sb", [MH, 1], f32).ap()
    MB3_sb = nc.alloc_sbuf_tensor("MB3_sb", [128, NCLS], f32).ap()

    z_full = nc.dram_tensor("z_full", [NPAD, DH], bf16, kind="Internal",
                            addr_space="Shared").ap()
    z4_full = nc.dram_tensor("z4_full", [NPAD, 128], bf16, kind="Internal",
                             addr_space="Shared").ap()

    rg = [list(range(NCORES))]

    with tile.TileContext(nc) as tc:
        with (
            tc.tile_pool(name="dram", bufs=1, space="DRAM") as dram,
            tc.tile_pool(name="meta", bufs=3) as meta,
            tc.tile_pool(name="gp", bufs=2) as gp,
            tc.tile_pool(name="sp", bufs=4) as sp,
            tc.tile_pool(name="zp", bufs=3) as zp,
            tc.tile_pool(name="hp", bufs=3) as hp,
            tc.tile_pool(name="ps", bufs=2, space="PSUM") as ps,
            tc.tile_pool(name="psa", bufs=2, space="PSUM") as psa,
            tc.tile_pool(name="pst", bufs=2, space="PSUM") as pst,
        ):
            z_loc = dram.tile([SHARD, DH], bf16)
            z4_loc = dram.tile([SHARD, 128], bf16)

            # load constants / weights
            for k in range(KC):
                nc.sync.dma_start(hT[k][:, :], xT[k * 128:(k + 1) * 128, :])
            nc.sync.dma_start(iota_sb, iota_c.ap())
            nc.sync.dma_start(ident_sb, ident_c.ap())
            nc.sync.dma_start(W4_sb, Wd[4].ap())
            for l in (1, 2, 3, 4):
                nc.sync.dma_start(Bb_sb[l], Bd[l].ap())
            nc.sync.dma_start(M1_sb, M1p.ap())
            nc.sync.dma_start(M2_sb, M2d.ap())
            nc.sync.dma_start(M3_sb, M3d.ap())
            nc.sync.dma_start(MB1_sb, MB1.ap())
            nc.sync.dma_start(MB2_sb, MB2.ap())
            nc.sync.dma_start(MB3_sb, MB3b.ap())

            for l in (1, 2, 3, 4):
                Dout = DH if l < 4 else 128
                zl = z_loc if l < 4 else z4_loc
                zf = z_full if l < 4 else z4_full
                wsb = W_sb if l < 4 else W4_sb
                if l < 4:
                    nc.sync.dma_start(W_sb, Wd[l].ap())

                # ---- dense: z = h @ W_l  (node-major out, bf16 to DRAM)
                for t in range(NB):
                    zps = ps.tile([128, DH], f32, tag="zps")
                    for k in range(KC):
                        nc.tensor.matmul(
                            zps[:, 0:Dout],
                            hT[k][:, t * 128:(t + 1) * 128],
                            wsb[:, k * Dout:(k + 1) * Dout],
                            start=(k == 0), stop=(k == KC - 1),
                        )
                    zsb = zp.tile([128, Dout], bf16, tag="zsb")
                    nc.vector.tensor_copy(zsb[:], zps[:, 0:Dout])
                    nc.sync.dma_start(zl[t * 128:(t + 1) * 128, :], zsb[:])

                # ---- allgather full z
                nc.gpsimd.collective_compute(
                    "AllGather", mybir.AluOpType.bypass,
                    replica_groups=rg, ins=[zl.opt()], outs=[zf],
                )

                # ---- aggregate per dst block
                col = 0
                for b in range(NB):
                    Tlo, Thi = int(T_seg[b, 0]), int(T_seg[b, 1])
                    Tb = Tlo + Thi
                    gbuf = gp.tile([128, TBMAX, Dout], bf16, tag="gbuf")
                    idx_sb = meta.tile([128, Tb * 8], i16, tag="idx")
                    nc.sync.dma_start(idx_sb[:], gidx.ap()[:, col * 8:(col + Tb) * 8])
                    dst_sb = meta.tile([128, Tb], f32, tag="dst")
                    nc.sync.dma_start(dst_sb[:], dstc.ap()[:, col:col + Tb])
                    w_sb = meta.tile([128, Tb], f32, tag="w")
                    nc.sync.dma_start(w_sb[:], wc.ap()[:, col:col + Tb])

                    # SWDGE descriptor ring holds ~1024 descs; cap each
                    # gather call at 7 tiles (896 idxs) to fit.
                    off = 0
                    for (Tg, base) in ((Tlo, 0), (Thi, SPLIT)):
                        left = Tg
                        while left > 0:
                            sub = min(7, left)
                            n = sub * 128
                            nc.gpsimd.dma_gather(
                                gbuf[:, off:off + sub, :],
                                zf[base:base + (SPLIT if base == 0 else NPAD - SPLIT), :],
                                idx_sb[:, off * 8:(off + sub) * 8],
                                num_idxs=n, num_idxs_reg=n, elem_size=Dout,
                            )
                            off += sub
                            left -= sub

                    aps = psa.tile([128, DH], f32, tag="aps")
                    for e in range(Tb):
                        S = sp.tile([128, 128], bf16, tag="S")
                        nc.vector.tensor_scalar(
                            S[:], iota_sb, dst_sb[:, e:e + 1], w_sb[:, e:e + 1],
                            mybir.AluOpType.is_equal, mybir.AluOpType.mult,
                        )
                        nc.tensor.matmul(
                            aps[:, 0:Dout], S[:], gbuf[:, e, :],
                            start=(e == 0), stop=(e == Tb - 1),
                        )

                    # epilogue: h = relu(agg + b); transpose back to feature-major
                    hsb = hp.tile([128, Dout], f32, tag="hsb")
                    nc.vector.tensor_tensor(
                        hsb[:], aps[:, 0:Dout], Bb_sb[l][:, 0:Dout],
                        mybir.AluOpType.add,
                    )
                    nc.scalar.activation(hsb[:], hsb[:],
                                         mybir.ActivationFunctionType.Relu)
                    for k in range(Dout // 128):
                        tps = pst.tile([128, 128], f32, tag="tps")
                        nc.tensor.transpose(tps[:], hsb[:, k * 128:(k + 1) * 128],
                                            ident_sb)
                        nc.vector.tensor_copy(hT[k][:, b * 128:(b + 1) * 128], tps[:])
                    col += Tb

            # ---- MLP head (feature-major until the last matmul)
            for c0 in range(0, SHARD, 512):
                wdt = min(512, SHARD - c0)
                p5 = ps.tile([MH, 512], f32, tag="zps")
                nc.tensor.matmul(p5[:, 0:wdt], M1_sb, hT[0][:, c0:c0 + wdt],
                                 start=True, stop=True)
                h5 = hp.tile([MH, 512], f32r, tag="h5")
                nc.scalar.activation(h5[:, 0:wdt], p5[:, 0:wdt],
                                     mybir.ActivationFunctionType.Relu, bias=MB1_sb)
                p6 = psa.tile([MH, 512], f32, tag="aps")
                nc.tensor.matmul(p6[:, 0:wdt], M2_sb, h5[:, 0:wdt],
                                 start=True, stop=True)
                h6 = hp.tile([MH, 512], f32r, tag="h6")
                nc.scalar.activation(h6[:, 0:wdt], p6[:, 0:wdt],
                                     mybir.ActivationFunctionType.Relu, bias=MB2_sb)
                for i in range(wdt // 128):
                    po = pst.tile([128, NCLS], f32, tag="tps")
                    nc.tensor.matmul(po[:], h6[:, i * 128:(i + 1) * 128], M3_sb,
                                     start=True, stop=True)
                    osb = zp.tile([128, NCLS], f32, tag="osb")
                    nc.vector.tensor_tensor(osb[:], po[:], MB3_sb,
                                            mybir.AluOpType.add)
                    nc.sync.dma_start(
                        out.ap()[c0 + i * 128:c0 + (i + 1) * 128, :], osb[:])

    nc.compile()
    return nc


# ---------------------------------------------------------------- entry point

def kernel(x, edge_index, W1, b1, W2, b2, W3, b3, W4, b4,
           M1, mb1, M2, mb2, M3, mb3):
    global LAST_RESULT
    x = np.asarray(x, np.float32)
    edge_index = np.asarray(edge_index)
    meta = _host_prep(x, edge_index)
    key = (tuple(meta["T_seg"].reshape(-1).tolist()), meta["TT"])
    if key not in _BUILD_CACHE:
        _BUILD_CACHE[key] = _build(key[0], key[1])
    nc = _BUILD_CACHE[key]

    W4p = np.zeros((DIN, 128), np.float32)
    W4p[:, :DE] = np.asarray(W4, np.float32)
    b4p = np.zeros(128, np.float32)
    b4p[:DE] = np.asarray(b4, np.float32)
    M1p = np.zeros((128, MH), np.float32)
    M1p[:DE] = np.asarray(M1, np.float32)

    Wch = {1: _chunk_w(np.asarray(W1, np.float32)),
           2: _chunk_w(np.asarray(W2, np.float32)),
           3: _chunk_w(np.asarray(W3, np.float32)),
           4: _chunk_w(W4p)}
    Bb = {1: np.broadcast_to(np.asarray(b1, np.float32), (128, DH)).copy(),
          2: np.broadcast_to(np.asarray(b2, np.float32), (128, DH)).copy(),
          3: np.broadcast_to(np.asarray(b3, np.float32), (128, DH)).copy(),
          4: np.broadcast_to(b4p, (128, 128)).copy()}

    common = {
        **{f"W{l}": Wch[l] for l in (1, 2, 3, 4)},
        **{f"B{l}": Bb[l] for l in (1, 2, 3, 4)},
        "M1p": M1p.astype(np.float32),
        "M2d": np.asarray(M2, np.float32),
        "M3d": np.asarray(M3, np.float32),
        "MB1": np.asarray(mb1, np.float32).reshape(MH, 1),
        "MB2": np.asarray(mb2, np.float32).reshape(MH, 1),
        "MB3b": np.broadcast_to(np.asarray(mb3, np.float32), (128, NCLS)).copy(),
        "iota_c": np.broadcast_to(np.arange(128, dtype=np.float32), (128, 128)).copy(),
        "ident_c": np.eye(128, dtype=np.float32),
    }

    in_maps = []
    for r in range(NCORES):
        rows = min(SHARD, max(0, N - r * SHARD))
        xp = np.zeros((SHARD, DIN), np.float32)
        xp[:rows] = x[r * SHARD:r * SHARD + rows]
        in_maps.append({
            **common,
            "xT": np.ascontiguousarray(xp.T),
            "gidx": meta["gidx"][r],
            "dstc": meta["dstc"][r],
            "wc": meta["wc"][r],
        })

    LAST_RESULT = bass_utils.run_bass_kernel_spmd(
        nc, in_maps, core_ids=list(range(NCORES)),
    )
    out = np.concatenate([LAST_RESULT.results[r]["out"] for r in range(NCORES)], 0)
    return np.ascontiguousarray(out[:N]).astype(np.float32)


# revision 6
# speedup vs baseline: 1.2923x; 1.2923x over previous
"""Trainium2 Bass kernel for nn_DomainGCN (4-layer GCN + MLP head), 8 NeuronCores.

Strategy (graph/data parallel, per sharding hint):
  - Nodes sharded contiguously: core r owns rows [r*6272, (r+1)*6272) (padded).
  - Reformulation: Agg(h@W) with per-edge weight dis[src]*dis[dst] and
    self-loops folded in as extra edges (weight dis[j]^2), so each layer is
      z = h @ W            (dense, fp32r matmuls, feature-major h resident in SBUF)
      z -> bf16 -> AllGather (every core gets the full z, node-major, in DRAM)
      agg = S.T @ z[src]   (dma_gather of edge messages + one-hot-weighted
                            matmuls accumulating per 128-dst-node block in PSUM)
      h' = relu(agg + b)   (DVE add + ACT relu), PE-transposed back to
                            feature-major for the next dense matmul.
  - Edge bookkeeping (sort by dst block / src range, degree rsqrt, padding) is
    host-side index preprocessing; all FLOP-bearing math runs on device.
  - dma_gather indices are int16, so each block's edge list is split at
    src=32768 and gathered from a base-offset slice of the gathered z.
"""

import os
import math
import numpy as np

import concourse.bass as bass
import concourse.bacc as bacc
import concourse.mybir as mybir
import concourse.tile as tile
from concourse import bass_utils

# problem constants (hardcoded per task contract)
N, E = 50000, 800000
DIN = DH = 512
DE, MH, NCLS = 10, 64, 20
NCORES = 8
NB = 49                   # dst blocks of 128 nodes per core
SHARD = NB * 128          # 6272
NPAD = SHARD * NCORES     # 50176
SPLIT = 32768             # int16 gather-index boundary
KC = 4                    # 128-feature chunks of 512

f32 = mybir.dt.float32
f32r = mybir.dt.float32r
bf16 = mybir.dt.bfloat16
i16 = mybir.dt.int16

LAST_RESULT = None        # BassKernelResults of the most recent run (for test.py)
_BUILD_CACHE = {}


# ---------------------------------------------------------------- host prep

def _host_prep(x, edge_index):
    src = edge_index[0].astype(np.int64)
    dst = edge_index[1].astype(np.int64)
    deg = np.bincount(dst, minlength=N).astype(np.float32) + 1.0
    dis = (1.0 / np.sqrt(deg)).astype(np.float32)

    # augment with self loops; uniform edge weight dis[src]*dis[dst]
    sa = np.concatenate([src, np.arange(N, dtype=np.int64)])
    da = np.concatenate([dst, np.arange(N, dtype=np.int64)])
    w = (dis[sa] * dis[da]).astype(np.float32)

    r = da // SHARD
    b = (da % SHARD) // 128
    s = (sa >= SPLIT).astype(np.int64)
    order = np.lexsort((sa, s, b, r))
    sa, da, w = sa[order], da[order], w[order]
    key = (r * NB + b) * 2 + s
    ks = key[order]
    counts = np.bincount(ks, minlength=NCORES * NB * 2).reshape(NCORES, NB, 2)
    starts = np.zeros(NCORES * NB * 2 + 1, np.int64)
    np.cumsum(counts.reshape(-1), out=starts[1:])

    # uniform program structure: per (block, seg) tile count = max over cores
    T_seg = np.ceil(counts / 128.0).astype(np.int64).max(axis=0)  # [NB, 2]
    TT = int(T_seg.sum())

    import ml_dtypes
    gidx, Sh = [], []
    for rr in range(NCORES):
        idx16 = np.zeros(TT * 128, np.int16)
        dcol = np.full(TT * 128, -1, np.int64)
        wcol = np.zeros(TT * 128, np.float32)
        cur = 0
        for bb in range(NB):
            for ss in range(2):
                k = (rr * NB + bb) * 2 + ss
                n = counts[rr, bb, ss]
                lo, hi = starts[k], starts[k] + n
                base = SPLIT if ss else 0
                idx16[cur:cur + n] = (sa[lo:hi] - base).astype(np.int16)
                dcol[cur:cur + n] = da[lo:hi] - rr * SHARD - bb * 128
                wcol[cur:cur + n] = w[lo:hi]
                cur += int(T_seg[bb, ss]) * 128   # pads: idx 0, dst -1, w 0
        gidx.append(np.tile(idx16.reshape(-1, 16).T, (8, 1)))       # [128, TT*8]
        # S tiles, host-precomputed: tile e is [128 edges, 128 dst] bf16 with
        # S[p, d] = w[e*128+p] iff dst_local[e*128+p] == d.
        A = np.zeros((TT * 128, 128), np.float32)
        valid = dcol >= 0
        A[np.nonzero(valid)[0], dcol[valid]] = wcol[valid]
        A = A.astype(ml_dtypes.bfloat16).reshape(TT, 128, 128)
        Sh.append(np.ascontiguousarray(A.transpose(1, 0, 2).reshape(128, TT * 128)))

    return {
        "T_seg": T_seg, "TT": TT, "gidx": gidx, "Sh": Sh,
    }


def _chunk_w(W):
    """[K, M] -> [128, (K//128)*M] with k-chunk c at cols [c*M, (c+1)*M)."""
    K, M = W.shape
    return np.ascontiguousarray(
        W.reshape(K // 128, 128, M).transpose(1, 0, 2).reshape(128, -1)
    ).astype(np.float32)


# ---------------------------------------------------------------- kernel build

def _build(T_seg_t, TT):
    T_seg = np.asarray(T_seg_t).reshape(NB, 2)
    TBMAX = int((T_seg[:, 0] + T_seg[:, 1]).max())

    nc = bacc.Bacc("TRN2", target_bir_lowering=False, debug=False,
                   num_devices=NCORES, num_swdge_queues=4)

    dt_in = {}

    def din(name, shape, dt):
        dt_in[name] = nc.dram_tensor(name, shape, dt, kind="ExternalInput")
        return dt_in[name]

    xT = din("xT", [DIN, SHARD], f32r)
    Wd = {l: din(f"W{l}", [128, KC * (DH if l < 4 else 128)], f32r) for l in (1, 2, 3, 4)}
    Bd = {l: din(f"B{l}", [128, DH if l < 4 else 128], f32) for l in (1, 2, 3, 4)}
    M1p = din("M1p", [128, MH], f32r)
    M2d = din("M2d", [MH, MH], f32r)
    M3d = din("M3d", [MH, NCLS], f32r)
    MB1 = din("MB1", [MH, 1], f32)
    MB2 = din("MB2", [MH, 1], f32)
    MB3b = din("MB3b", [128, NCLS], f32)
    ident_c = din("ident_c", [128, 128], f32)
    gidx = din("gidx", [128, TT * 8], i16)
    Sh = din("Sh", [128, TT * 128], bf16)
    out = nc.dram_tensor("out", [SHARD, NCLS], f32, kind="ExternalOutput")

    # persistent SBUF
    hT = [nc.alloc_sbuf_tensor(f"hT{k}", [128, SHARD], f32r).ap() for k in range(KC)]
    W_sb = nc.alloc_sbuf_tensor("W_sb", [128, KC * DH], f32r).ap()
    W4_sb = nc.alloc_sbuf_tensor("W4_sb", [128, KC * 128], f32r).ap()
    ident_sb = nc.alloc_sbuf_tensor("ident_sb", [128, 128], f32).ap()
    Bb_sb = {l: nc.alloc_sbuf_tensor(f"Bb{l}", [128, DH if l < 4 else 128], f32).ap()
             for l in (1, 2, 3, 4)}
    M1_sb = nc.alloc_sbuf_tensor("M1_sb", [128, MH], f32r).ap()
    M2_sb = nc.alloc_sbuf_tensor("M2_sb", [MH, MH], f32r).ap()
    M3_sb = nc.alloc_sbuf_tensor("M3_sb", [MH, NCLS], f32r).ap()
    MB1_sb = nc.alloc_sbuf_tensor("MB1_sb", [MH, 1], f32).ap()
    MB2_sb = nc.alloc_sbuf_tensor("MB2_sb", [MH, 1], f32).ap()
    MB3_sb = nc.alloc_sbuf_tensor("MB3_sb", [128, NCLS], f32).ap()

    z_full = nc.dram_tensor("z_full", [NPAD, DH], bf16, kind="Internal",
                            addr_space="Shared").ap()
    z4_full = nc.dram_tensor("z4_full", [NPAD, 128], bf16, kind="Internal",
                             addr_space="Shared").ap()

    rg = [list(range(NCORES))]

    with tile.TileContext(nc) as tc:
        with (
            tc.tile_pool(name="dram", bufs=1, space="DRAM") as dram,
            tc.tile_pool(name="meta", bufs=3) as meta,
            tc.tile_pool(name="gp", bufs=2) as gp,
            tc.tile_pool(name="sp", bufs=4) as sp,
            tc.tile_pool(name="zp", bufs=3) as zp,
            tc.tile_pool(name="hp", bufs=3) as hp,
            tc.tile_pool(name="ps", bufs=2, space="PSUM") as ps,
            tc.tile_pool(name="psa", bufs=2, space="PSUM") as psa,
            tc.tile_pool(name="pst", bufs=2, space="PSUM") as pst,
        ):
            z_loc = dram.tile([SHARD, DH], bf16)
            z4_loc = dram.tile([SHARD, 128], bf16)

            # load constants / weights
            for k in range(KC):
                nc.sync.dma_start(hT[k][:, :], xT[k * 128:(k + 1) * 128, :])
            nc.sync.dma_start(ident_sb, ident_c.ap())
            nc.sync.dma_start(W4_sb, Wd[4].ap())
            for l in (1, 2, 3, 4):
                nc.sync.dma_start(Bb_sb[l], Bd[l].ap())
            nc.sync.dma_start(M1_sb, M1p.ap())
            nc.sync.dma_start(M2_sb, M2d.ap())
            nc.sync.dma_start(M3_sb, M3d.ap())
            nc.sync.dma_start(MB1_sb, MB1.ap())
            nc.sync.dma_start(MB2_sb, MB2.ap())
            nc.sync.dma_start(MB3_sb, MB3b.ap())

            for l in (1, 2, 3, 4):
                Dout = DH if l < 4 else 128
                zl = z_loc if l < 4 else z4_loc
                zf = z_full if l < 4 else z4_full
                wsb = W_sb if l < 4 else W4_sb
                if l < 4:
                    nc.sync.dma_start(W_sb, Wd[l].ap())

                # ---- dense: z = h @ W_l  (node-major out, bf16 to DRAM)
                for t in range(NB):
                    zps = ps.tile([128, DH], f32, tag="zps")
                    for k in range(KC):
                        nc.tensor.matmul(
                            zps[:, 0:Dout],
                            hT[k][:, t * 128:(t + 1) * 128],
                            wsb[:, k * Dout:(k + 1) * Dout],
                            start=(k == 0), stop=(k == KC - 1),
                        )
                    zsb = zp.tile([128, Dout], bf16, tag="zsb")
                    nc.vector.tensor_copy(zsb[:], zps[:, 0:Dout])
                    nc.sync.dma_start(zl[t * 128:(t + 1) * 128, :], zsb[:])

                # ---- allgather full z
                nc.gpsimd.collective_compute(
                    "AllGather", mybir.AluOpType.bypass,
                    replica_groups=rg, ins=[zl.opt()], outs=[zf],
                )

                # ---- aggregate per dst block
                col = 0
                qn = 0
                for b in range(NB):
                    Tlo, Thi = int(T_seg[b, 0]), int(T_seg[b, 1])
                    Tb = Tlo + Thi
                    gbuf = gp.tile([128, TBMAX, Dout], bf16, tag="gbuf")
                    idx_sb = meta.tile([128, Tb * 8], i16, tag="idx")
                    nc.sync.dma_start(idx_sb[:], gidx.ap()[:, col * 8:(col + Tb) * 8])
                    S_sb = sp.tile([128, Tb, 128], bf16, tag="S")
                    nc.sync.dma_start(
                        S_sb[:],
                        Sh.ap()[:, col * 128:(col + Tb) * 128]
                        .rearrange("p (t d) -> p t d", t=Tb))

                    # SWDGE descriptor ring holds ~1024 descs; cap each
                    # gather call at 7 tiles (896 idxs) to fit.
                    off = 0
                    for (Tg, base) in ((Tlo, 0), (Thi, SPLIT)):
                        left = Tg
                        while left > 0:
                            sub = min(7, left)
                            n = sub * 128
                            nc.gpsimd.dma_gather(
                                gbuf[:, off:off + sub, :],
                                zf[base:base + (SPLIT if base == 0 else NPAD - SPLIT), :],
                                idx_sb[:, off * 8:(off + sub) * 8],
                                num_idxs=n, num_idxs_reg=n, elem_size=Dout,
                                queue_num=qn,
                            )
                            qn = (qn + 1) % 4
                            off += sub
                            left -= sub

                    aps = psa.tile([128, DH], f32, tag="aps")
                    for e in range(Tb):
                        nc.tensor.matmul(
                            aps[:, 0:Dout], S_sb[:, e, :], gbuf[:, e, :],
                            start=(e == 0), stop=(e == Tb - 1),
                        )

                    # epilogue: h = relu(agg + b); transpose back to feature-major
                    hsb = hp.tile([128, Dout], f32, tag="hsb")
                    nc.vector.tensor_tensor(
                        hsb[:], aps[:, 0:Dout], Bb_sb[l][:, 0:Dout],
                        mybir.AluOpType.add,
                    )
                    nc.scalar.activation(hsb[:], hsb[:],
                                         mybir.ActivationFunctionType.Relu)
                    for k in range(Dout // 128):
                        tps = pst.tile([128, 128], f32, tag="tps")
                        nc.tensor.transpose(tps[:], hsb[:, k * 128:(k + 1) * 128],
                                            ident_sb)
                        nc.vector.tensor_copy(hT[k][:, b * 128:(b + 1) * 128], tps[:])
                    col += Tb

            # ---- MLP head (feature-major until the last matmul)
            for c0 in range(0, SHARD, 512):
                wdt = min(512, SHARD - c0)
                p5 = ps.tile([MH, 512], f32, tag="zps")
                nc.tensor.matmul(p5[:, 0:wdt], M1_sb, hT[0][:, c0:c0 + wdt],
                                 start=True, stop=True)
                h5 = hp.tile([MH, 512], f32r, tag="h5")
                nc.scalar.activation(h5[:, 0:wdt], p5[:, 0:wdt],
                                     mybir.ActivationFunctionType.Relu, bias=MB1_sb)
                p6 = psa.tile([MH, 512], f32, tag="aps")
                nc.tensor.matmul(p6[:, 0:wdt], M2_sb, h5[:, 0:wdt],
                                 start=True, stop=True)
                h6 = hp.tile([MH, 512], f32r, tag="h6")
                nc.scalar.activation(h6[:, 0:wdt], p6[:, 0:wdt],
                                     mybir.ActivationFunctionType.Relu, bias=MB2_sb)
                for i in range(wdt // 128):
                    po = pst.tile([128, NCLS], f32, tag="tps")
                    nc.tensor.matmul(po[:], h6[:, i * 128:(i + 1) * 128], M3_sb,
                                     start=True, stop=True)
                    osb = zp.tile([128, NCLS], f32, tag="osb")
                    nc.vector.tensor_tensor(osb[:], po[:], MB3_sb,
                                            mybir.AluOpType.add)
                    nc.sync.dma_start(
                        out.ap()[c0 + i * 128:c0 + (i + 1) * 128, :], osb[:])

    nc.compile()
    return nc


# ---------------------------------------------------------------- entry point

def kernel(x, edge_index, W1, b1, W2, b2, W3, b3, W4, b4,
           M1, mb1, M2, mb2, M3, mb3):
    global LAST_RESULT
    x = np.asarray(x, np.float32)
    edge_index = np.asarray(edge_index)
    meta = _host_prep(x, edge_index)
    key = (tuple(meta["T_seg"].reshape(-1).tolist()), meta["TT"])
    if key not in _BUILD_CACHE:
        _BUILD_CACHE[key] = _build(key[0], key[1])
    nc = _BUILD_CACHE[key]

    W4p = np.zeros((DIN, 128), np.float32)
    W4p[:, :DE] = np.asarray(W4, np.float32)
    b4p = np.zeros(128, np.float32)
    b4p[:DE] = np.asarray(b4, np.float32)
    M1p = np.zeros((128, MH), np.float32)
    M1p[:DE] = np.asarray(M1, np.float32)

    Wch = {1: _chunk_w(np.asarray(W1, np.float32)),
           2: _chunk_w(np.asarray(W2, np.float32)),
           3: _chunk_w(np.asarray(W3, np.float32)),
           4: _chunk_w(W4p)}
    Bb = {1: np.broadcast_to(np.asarray(b1, np.float32), (128, DH)).copy(),
          2: np.broadcast_to(np.asarray(b2, np.float32), (128, DH)).copy(),
          3: np.broadcast_to(np.asarray(b3, np.float32), (128, DH)).copy(),
          4: np.broadcast_to(b4p, (128, 128)).copy()}

    common = {
        **{f"W{l}": Wch[l] for l in (1, 2, 3, 4)},
        **{f"B{l}": Bb[l] for l in (1, 2, 3, 4)},
        "M1p": M1p.astype(np.float32),
        "M2d": np.asarray(M2, np.float32),
        "M3d": np.asarray(M3, np.float32),
        "MB1": np.asarray(mb1, np.float32).reshape(MH, 1),
        "MB2": np.asarray(mb2, np.float32).reshape(MH, 1),
        "MB3b": np.broadcast_to(np.asarray(mb3, np.float32), (128, NCLS)).copy(),
        "ident_c": np.eye(128, dtype=np.float32),
    }

    in_maps = []
    for r in range(NCORES):
        rows = min(SHARD, max(0, N - r * SHARD))
        xp = np.zeros((SHARD, DIN), np.float32)
        xp[:rows] = x[r * SHARD:r * SHARD + rows]
        in_maps.append({
            **common,
            "xT": np.ascontiguousarray(xp.T),
            "gidx": meta["gidx"][r],
            "Sh": meta["Sh"][r],
        })

    LAST_RESULT = bass_utils.run_bass_kernel_spmd(
        nc, in_maps, core_ids=list(range(NCORES)),
    )
    out = np.concatenate([LAST_RESULT.results[r]["out"] for r in range(NCORES)], 0)
    return np.ascontiguousarray(out[:N]).astype(np.float32)


# revision 9
# speedup vs baseline: 1.5478x; 1.1977x over previous
"""Trainium2 Bass kernel for nn_DomainGCN (4-layer GCN + MLP head), 8 NeuronCores.

Strategy (graph/data parallel, per sharding hint):
  - Nodes sharded contiguously: core r owns rows [r*6272, (r+1)*6272) (padded).
  - Reformulation: Agg(h@W) with per-edge weight dis[src]*dis[dst] and
    self-loops folded in as extra edges (weight dis[j]^2), so each layer is
      z = h @ W            (dense, fp32r matmuls, feature-major h resident in SBUF)
      z -> bf16 -> AllGather (every core gets the full z, node-major, in DRAM)
      agg = S.T @ z[src]   (dma_gather of edge messages + one-hot-weighted
                            matmuls accumulating per 128-dst-node block in PSUM)
      h' = relu(agg + b)   (DVE add + ACT relu), PE-transposed back to
                            feature-major for the next dense matmul.
  - Edge bookkeeping (sort by dst block / src range, degree rsqrt, padding) is
    host-side index preprocessing; all FLOP-bearing math runs on device.
  - dma_gather indices are int16, so each block's edge list is split at
    src=32768 and gathered from a base-offset slice of the gathered z.
"""

import os
import math
import numpy as np

import concourse.bass as bass
import concourse.bacc as bacc
import concourse.mybir as mybir
import concourse.tile as tile
from concourse import bass_utils

# problem constants (hardcoded per task contract)
N, E = 50000, 800000
DIN = DH = 512
DE, MH, NCLS = 10, 64, 20
NCORES = 8
NB = 49                   # dst blocks of 128 nodes per core
SHARD = NB * 128          # 6272
NPAD = SHARD * NCORES     # 50176
SPLIT = 32768             # int16 gather-index boundary
KC = 4                    # 128-feature chunks of 512

f32 = mybir.dt.float32
f32r = mybir.dt.float32r
bf16 = mybir.dt.bfloat16
i16 = mybir.dt.int16

LAST_RESULT = None        # BassKernelResults of the most recent run (for test.py)
_BUILD_CACHE = {}


# ---------------------------------------------------------------- host prep

def _host_prep(x, edge_index):
    src = edge_index[0].astype(np.int64)
    dst = edge_index[1].astype(np.int64)
    deg = np.bincount(dst, minlength=N).astype(np.float32) + 1.0
    dis = (1.0 / np.sqrt(deg)).astype(np.float32)

    # Real edges only; self loops are handled as a dedicated per-block
    # "self tile" fed by a contiguous DMA from the local z (no gather).
    sa, da = src, dst
    w = (dis[sa] * dis[da]).astype(np.float32)

    # Segments: src shard-half membership.  AllGather runs as two half
    # collectives (rows [0,HALF) of every shard, then [HALF,SHARD)), so the
    # gathered tensors are z_fullA/z_fullB with rank-major halves; row ids
    # stay < 32768 (int16-safe) with no extra split.
    HALF = SHARD // 2  # 3136
    r = da // SHARD
    b = (da % SHARD) // 128
    so = sa % SHARD
    s = (so >= HALF).astype(np.int64)
    row = (sa // SHARD) * HALF + (so - s * HALF)   # row in z_fullA or z_fullB
    order = np.lexsort((row, s, b, r))
    row_s, da_s, w_s = row[order], da[order], w[order]
    key = (r * NB + b) * 2 + s
    ks = key[order]
    counts = np.bincount(ks, minlength=NCORES * NB * 2).reshape(NCORES, NB, 2)
    starts = np.zeros(NCORES * NB * 2 + 1, np.int64)
    np.cumsum(counts.reshape(-1), out=starts[1:])

    # uniform program structure: per (block, seg) tile count = max over cores
    T_seg = np.ceil(counts / 128.0).astype(np.int64).max(axis=0)  # [NB, 2]
    TT = int(T_seg.sum()) + NB   # +1 self tile per block

    import ml_dtypes
    gidx, Sh = [], []
    for rr in range(NCORES):
        idx16 = np.zeros(TT * 128, np.int16)
        dcol = np.full(TT * 128, -1, np.int64)
        wcol = np.zeros(TT * 128, np.float32)
        cur = 0
        for bb in range(NB):
            # self tile (tile 0 of each block): S = diag(dis^2) of the
            # block's own nodes; msg row p comes from local z row bb*128+p.
            gnode = rr * SHARD + bb * 128 + np.arange(128)
            ok = gnode < N
            dcol[cur:cur + 128][ok] = np.arange(128)[ok]
            wcol[cur:cur + 128][ok] = (dis[gnode[ok]] ** 2)
            cur += 128
            for ss in range(2):
                k = (rr * NB + bb) * 2 + ss
                n = counts[rr, bb, ss]
                lo, hi = starts[k], starts[k] + n
                idx16[cur:cur + n] = row_s[lo:hi].astype(np.int16)
                dcol[cur:cur + n] = da_s[lo:hi] - rr * SHARD - bb * 128
                wcol[cur:cur + n] = w_s[lo:hi]
                cur += int(T_seg[bb, ss]) * 128   # pads: idx 0, dst -1, w 0
        gidx.append(np.tile(idx16.reshape(-1, 16).T, (8, 1)))       # [128, TT*8]
        # S tiles, host-precomputed: tile e is [128 edges, 128 dst] bf16 with
        # S[p, d] = w[e*128+p] iff dst_local[e*128+p] == d.
        A = np.zeros((TT * 128, 128), np.float32)
        valid = dcol >= 0
        A[np.nonzero(valid)[0], dcol[valid]] = wcol[valid]
        A = A.astype(ml_dtypes.bfloat16).reshape(TT, 128, 128)
        Sh.append(np.ascontiguousarray(A.transpose(1, 0, 2).reshape(128, TT * 128)))

    return {
        "T_seg": T_seg, "TT": TT, "gidx": gidx, "Sh": Sh,
    }


def _chunk_w(W):
    """[K, M] -> [128, (K//128)*M] with k-chunk c at cols [c*M, (c+1)*M)."""
    K, M = W.shape
    return np.ascontiguousarray(
        W.reshape(K // 128, 128, M).transpose(1, 0, 2).reshape(128, -1)
    ).astype(np.float32)


# ---------------------------------------------------------------- kernel build

def _build(T_seg_t, TT):
    T_seg = np.asarray(T_seg_t).reshape(NB, 2)
    TBMAX = int((T_seg[:, 0] + T_seg[:, 1]).max()) + 1

    nc = bacc.Bacc("TRN2", target_bir_lowering=False, debug=False,
                   num_devices=NCORES, num_swdge_queues=4)

    dt_in = {}

    def din(name, shape, dt):
        dt_in[name] = nc.dram_tensor(name, shape, dt, kind="ExternalInput")
        return dt_in[name]

    xT = din("xT", [DIN, SHARD], f32r)
    Wd = {l: din(f"W{l}", [128, KC * (DH if l < 4 else 128)], f32r) for l in (1, 2, 3, 4)}
    Bd = {l: din(f"B{l}", [128, DH if l < 4 else 128], f32) for l in (1, 2, 3, 4)}
    M1p = din("M1p", [128, MH], f32r)
    M2d = din("M2d", [MH, MH], f32r)
    M3d = din("M3d", [MH, NCLS], f32r)
    MB1 = din("MB1", [MH, 1], f32)
    MB2 = din("MB2", [MH, 1], f32)
    MB3b = din("MB3b", [128, NCLS], f32)
    ident_c = din("ident_c", [128, 128], f32)
    gidx = din("gidx", [128, TT * 8], i16)
    Sh = din("Sh", [128, TT * 128], bf16)
    out = nc.dram_tensor("out", [SHARD, NCLS], f32, kind="ExternalOutput")

    # persistent SBUF
    hT = [nc.alloc_sbuf_tensor(f"hT{k}", [128, SHARD], f32r).ap() for k in range(KC)]
    W_sb = nc.alloc_sbuf_tensor("W_sb", [128, KC * DH], f32r).ap()
    W4_sb = nc.alloc_sbuf_tensor("W4_sb", [128, KC * 128], f32r).ap()
    ident_sb = nc.alloc_sbuf_tensor("ident_sb", [128, 128], f32).ap()
    Bb_sb = {l: nc.alloc_sbuf_tensor(f"Bb{l}", [128, DH if l < 4 else 128], f32).ap()
             for l in (1, 2, 3, 4)}
    M1_sb = nc.alloc_sbuf_tensor("M1_sb", [128, MH], f32r).ap()
    M2_sb = nc.alloc_sbuf_tensor("M2_sb", [MH, MH], f32r).ap()
    M3_sb = nc.alloc_sbuf_tensor("M3_sb", [MH, NCLS], f32r).ap()
    MB1_sb = nc.alloc_sbuf_tensor("MB1_sb", [MH, 1], f32).ap()
    MB2_sb = nc.alloc_sbuf_tensor("MB2_sb", [MH, 1], f32).ap()
    MB3_sb = nc.alloc_sbuf_tensor("MB3_sb", [128, NCLS], f32).ap()

    HALF = SHARD // 2
    NPH = NCORES * HALF
    zfA = nc.dram_tensor("zfA", [NPH, DH], bf16, kind="Internal",
                         addr_space="Shared").ap()
    zfB = nc.dram_tensor("zfB", [NPH, DH], bf16, kind="Internal",
                         addr_space="Shared").ap()
    z4fA = nc.dram_tensor("z4fA", [NPH, 128], bf16, kind="Internal",
                          addr_space="Shared").ap()
    z4fB = nc.dram_tensor("z4fB", [NPH, 128], bf16, kind="Internal",
                          addr_space="Shared").ap()

    rg = [list(range(NCORES))]

    with tile.TileContext(nc) as tc:
        with (
            tc.tile_pool(name="dram", bufs=1, space="DRAM") as dram,
            tc.tile_pool(name="meta", bufs=3) as meta,
            tc.tile_pool(name="gp", bufs=2) as gp,
            tc.tile_pool(name="sp", bufs=4) as sp,
            tc.tile_pool(name="zp", bufs=3) as zp,
            tc.tile_pool(name="hp", bufs=3) as hp,
            tc.tile_pool(name="ps", bufs=2, space="PSUM") as ps,
            tc.tile_pool(name="psa", bufs=2, space="PSUM") as psa,
            tc.tile_pool(name="pst", bufs=2, space="PSUM") as pst,
        ):
            z_loc = dram.tile([SHARD, DH], bf16)
            z4_loc = dram.tile([SHARD, 128], bf16)

            # load constants / weights
            for k in range(KC):
                nc.sync.dma_start(hT[k][:, :], xT[k * 128:(k + 1) * 128, :])
            nc.sync.dma_start(ident_sb, ident_c.ap())
            nc.sync.dma_start(W4_sb, Wd[4].ap())
            for l in (1, 2, 3, 4):
                nc.sync.dma_start(Bb_sb[l], Bd[l].ap())
            nc.sync.dma_start(M1_sb, M1p.ap())
            nc.sync.dma_start(M2_sb, M2d.ap())
            nc.sync.dma_start(M3_sb, M3d.ap())
            nc.sync.dma_start(MB1_sb, MB1.ap())
            nc.sync.dma_start(MB2_sb, MB2.ap())
            nc.sync.dma_start(MB3_sb, MB3b.ap())

            qn = 0
            for l in (1, 2, 3, 4):
                Dout = DH if l < 4 else 128
                zl = z_loc if l < 4 else z4_loc
                zA, zB = (zfA, zfB) if l < 4 else (z4fA, z4fB)
                wsb = W_sb if l < 4 else W4_sb
                if l < 4:
                    nc.sync.dma_start(W_sb, Wd[l].ap())

                # ---- dense: z = h @ W_l (node-major, bf16 to DRAM), with the
                # half-shard AllGather fired as soon as its half is written so
                # it overlaps the rest of the dense phase / previous agg tail.
                for t in range(NB):
                    zps = ps.tile([128, DH], f32, tag="zps")
                    for k in range(KC):
                        nc.tensor.matmul(
                            zps[:, 0:Dout],
                            hT[k][:, t * 128:(t + 1) * 128],
                            wsb[:, k * Dout:(k + 1) * Dout],
                            start=(k == 0), stop=(k == KC - 1),
                        )
                    zsb = zp.tile([128, Dout], bf16, tag="zsb")
                    nc.vector.tensor_copy(zsb[:], zps[:, 0:Dout])
                    nc.sync.dma_start(zl[t * 128:(t + 1) * 128, :], zsb[:])
                    if t == NB // 2:  # rows [0, HALF) complete after tile 24
                        nc.gpsimd.collective_compute(
                            "AllGather", mybir.AluOpType.bypass,
                            replica_groups=rg, ins=[zl[0:HALF, :]], outs=[zA],
                        )
                nc.gpsimd.collective_compute(
                    "AllGather", mybir.AluOpType.bypass,
                    replica_groups=rg, ins=[zl[HALF:SHARD, :]], outs=[zB],
                )

                # ---- aggregate per dst block
                col = 0
                for b in range(NB):
                    TA, TB_ = int(T_seg[b, 0]), int(T_seg[b, 1])
                    Tb = 1 + TA + TB_
                    gbuf = gp.tile([128, TBMAX, Dout], bf16, tag="gbuf")
                    idx_sb = meta.tile([128, Tb * 8], i16, tag="idx")
                    nc.sync.dma_start(idx_sb[:], gidx.ap()[:, col * 8:(col + Tb) * 8])
                    S_sb = sp.tile([128, Tb, 128], bf16, tag="S")
                    nc.sync.dma_start(
                        S_sb[:],
                        Sh.ap()[:, col * 128:(col + Tb) * 128]
                        .rearrange("p (t d) -> p t d", t=Tb))

                    # self tile: contiguous copy of the block's own z rows
                    nc.sync.dma_start(gbuf[:, 0:1, :],
                                      zl[b * 128:(b + 1) * 128, :]
                                      .rearrange("(a p) d -> p a d", a=1))

                    # SWDGE descriptor ring holds ~1024 descs/queue; cap each
                    # gather call at 7 tiles (896 idxs).
                    off = 1
                    for (Tg, zsrc) in ((TA, zA), (TB_, zB)):
                        left = Tg
                        while left > 0:
                            sub = min(7, left)
                            n = sub * 128
                            nc.gpsimd.dma_gather(
                                gbuf[:, off:off + sub, :],
                                zsrc,
                                idx_sb[:, off * 8:(off + sub) * 8],
                                num_idxs=n, num_idxs_reg=n, elem_size=Dout,
                                queue_num=qn,
                            )
                            qn = (qn + 1) % 4
                            off += sub
                            left -= sub

                    aps = psa.tile([128, DH], f32, tag="aps")
                    for e in range(Tb):
                        nc.tensor.matmul(
                            aps[:, 0:Dout], S_sb[:, e, :], gbuf[:, e, :],
                            start=(e == 0), stop=(e == Tb - 1),
                        )

                    # epilogue: h = relu(agg + b); transpose back to feature-major
                    hsb = hp.tile([128, Dout], f32, tag="hsb")
                    nc.vector.tensor_tensor(
                        hsb[:], aps[:, 0:Dout], Bb_sb[l][:, 0:Dout],
                        mybir.AluOpType.add,
                    )
                    nc.scalar.activation(hsb[:], hsb[:],
                                         mybir.ActivationFunctionType.Relu)
                    for k in range(Dout // 128):
                        tps = pst.tile([128, 128], f32, tag="tps")
                        nc.tensor.transpose(tps[:], hsb[:, k * 128:(k + 1) * 128],
                                            ident_sb)
                        nc.vector.tensor_copy(hT[k][:, b * 128:(b + 1) * 128], tps[:])
                    col += Tb

            # ---- MLP head (feature-major until the last matmul)
            for c0 in range(0, SHARD, 512):
                wdt = min(512, SHARD - c0)
                p5 = ps.tile([MH, 512], f32, tag="zps")
                nc.tensor.matmul(p5[:, 0:wdt], M1_sb, hT[0][:, c0:c0 + wdt],
                                 start=True, stop=True)
                h5 = hp.tile([MH, 512], f32r, tag="h5")
                nc.scalar.activation(h5[:, 0:wdt], p5[:, 0:wdt],
                                     mybir.ActivationFunctionType.Relu, bias=MB1_sb)
                p6 = psa.tile([MH, 512], f32, tag="aps")
                nc.tensor.matmul(p6[:, 0:wdt], M2_sb, h5[:, 0:wdt],
                                 start=True, stop=True)
                h6 = hp.tile([MH, 512], f32r, tag="h6")
                nc.scalar.activation(h6[:, 0:wdt], p6[:, 0:wdt],
                                     mybir.ActivationFunctionType.Relu, bias=MB2_sb)
                for i in range(wdt // 128):
                    po = pst.tile([128, NCLS], f32, tag="tps")
                    nc.tensor.matmul(po[:], h6[:, i * 128:(i + 1) * 128], M3_sb,
                                     start=True, stop=True)
                    osb = zp.tile([128, NCLS], f32, tag="osb")
                    nc.vector.tensor_tensor(osb[:], po[:], MB3_sb,
                                            mybir.AluOpType.add)
                    nc.sync.dma_start(
                        out.ap()[c0 + i * 128:c0 + (i + 1) * 128, :], osb[:])

    nc.compile()
    return nc


# ---------------------------------------------------------------- entry point

def kernel(x, edge_index, W1, b1, W2, b2, W3, b3, W4, b4,
           M1, mb1, M2, mb2, M3, mb3):
    global LAST_RESULT
    x = np.asarray(x, np.float32)
    edge_index = np.asarray(edge_index)
    meta = _host_prep(x, edge_index)
    key = (tuple(meta["T_seg"].reshape(-1).tolist()), meta["TT"])
    if key not in _BUILD_CACHE:
        _BUILD_CACHE[key] = _build(key[0], key[1])
    nc = _BUILD_CACHE[key]

    W4p = np.zeros((DIN, 128), np.float32)
    W4p[:, :DE] = np.asarray(W4, np.float32)
    b4p = np.zeros(128, np.float32)
    b4p[:DE] = np.asarray(b4, np.float32)
    M1p = np.zeros((128, MH), np.float32)
    M1p[:DE] = np.asarray(M1, np.float32)

    Wch = {1: _chunk_w(np.asarray(W1, np.float32)),
           2: _chunk_w(np.asarray(W2, np.float32)),
           3: _chunk_w(np.asarray(W3, np.float32)),
           4: _chunk_w(W4p)}
    Bb = {1: np.broadcast_to(np.asarray(b1, np.float32), (128, DH)).copy(),
          2: np.broadcast_to(np.asarray(b2, np.float32), (128, DH)).copy(),
          3: np.broadcast_to(np.asarray(b3, np.float32), (128, DH)).copy(),
          4: np.broadcast_to(b4p, (128, 128)).copy()}

    common = {
        **{f"W{l}": Wch[l] for l in (1, 2, 3, 4)},
        **{f"B{l}": Bb[l] for l in (1, 2, 3, 4)},
        "M1p": M1p.astype(np.float32),
        "M2d": np.asarray(M2, np.float32),
        "M3d": np.asarray(M3, np.float32),
        "MB1": np.asarray(mb1, np.float32).reshape(MH, 1),
        "MB2": np.asarray(mb2, np.float32).reshape(MH, 1),
        "MB3b": np.broadcast_to(np.asarray(mb3, np.float32), (128, NCLS)).copy(),
        "ident_c": np.eye(128, dtype=np.float32),
    }

    in_maps = []
    for r in range(NCORES):
        rows = min(SHARD, max(0, N - r * SHARD))
        xp = np.zeros((SHARD, DIN), np.float32)
        xp[:rows] = x[r * SHARD:r * SHARD + rows]
        in_maps.append({
            **common,
            "xT": np.ascontiguousarray(xp.T),
            "gidx": meta["gidx"][r],
            "Sh": meta["Sh"][r],
        })

    LAST_RESULT = bass_utils.run_bass_kernel_spmd(
        nc, in_maps, core_ids=list(range(NCORES)),
    )
    out = np.concatenate([LAST_RESULT.results[r]["out"] for r in range(NCORES)], 0)
    return np.ascontiguousarray(out[:N]).astype(np.float32)
